# revision 1
# baseline (speedup 1.0000x reference)
"""Trainium2 Bass kernel for DeepSet segment-reduce problem.

Strategy: host reorders elements so that segments (contiguous CSR ranges) are
grouped into uniform-length classes; each segment is padded to its class
length by duplicating its own elements (max-invariant). The device program is
then fully regular: fp16 matmuls on PE, fused bias+LeakyReLU epilogues on the
scalar engine (Prelu activation), per-segment max as strided reduce_max on the
vector engine, and segment->element broadcast folded into the matmul rhs
access pattern (step-0 inner dim).

Self-contained: no reads of reference.py / spec.json.
"""
import numpy as np

import concourse.bass as bass
import concourse.mybir as mybir
import concourse.tile as tile
from concourse import bacc
from concourse.bass_utils import run_bass_kernel_spmd

N_CORES = 8
D_IN = 64
D_OUT = 128
ALPHA = 0.2
FD = 480                      # columns per chunk (one PSUM-bank's worth used)
GAP = 512                     # chunk stride inside a PSUM/SBUF group tile
GROUP_COLS = 2 * FD           # real columns per group (2 chunks)
CLASSES = [1, 2, 3, 4, 5, 6, 8, 10, 12, 15, 16, 20, 24, 30, 32, 40, 48]
LMAXC = 48                    # largest class; longer segments are split

F16 = mybir.dt.float16
F32 = mybir.dt.float32


# ----------------------------------------------------------------------------
# Host-side layout
# ----------------------------------------------------------------------------

def _next_class(lengths):
    """Smallest class >= length, vectorized."""
    cls = np.empty(len(lengths), dtype=np.int64)
    cls.fill(-1)
    for c in reversed(CLASSES):
        cls[lengths <= c] = c
    return cls


def build_layout(csr_idx):
    """Returns per-core element index map, orig map, and group structure.

    Output dict:
      elem_idx:  [n_cores, E] int64 -> index into x rows (0 for dummies)
      orig_of:   [n_cores, E] int64 -> original element id, or -1
      groups:    list of (L, n_fix0, n_fix1) per 960-col group (same all cores)
      p_max:     pieces per split segment (1 if none)
      E:         padded element count per core
    """
    csr = np.asarray(csr_idx).astype(np.int64)
    counts = csr[1:] - csr[:-1]
    nz = counts > 0
    seg_start = csr[:-1][nz]
    seg_len = counts[nz]
    lmax = int(seg_len.max()) if len(seg_len) else 1

    # --- split segments longer than LMAXC into P_MAX pieces of class LMAXC
    slots_per_chunk_48 = FD // LMAXC  # 10
    if lmax > LMAXC:
        for p in (2, 5, 10):
            if p * LMAXC >= lmax and slots_per_chunk_48 % p == 0:
                p_max = p
                break
        else:
            raise ValueError(f"segment too long: {lmax}")
    else:
        p_max = 1

    is_split = seg_len > LMAXC
    norm_start, norm_len = seg_start[~is_split], seg_len[~is_split]
    sp_start, sp_len = seg_start[is_split], seg_len[is_split]

    # --- round-robin split segments across cores; canonical count
    n_split_core = int(np.ceil(len(sp_start) / N_CORES)) if p_max > 1 else 0
    if p_max > 1:
        # pad so split slots fill whole halves (10/p_max fix-groups per half)
        per_half = slots_per_chunk_48 // p_max
        n_split_core = int(np.ceil(n_split_core / per_half)) * per_half

    # --- classes for normal segments
    cls = _next_class(norm_len)
    order = np.argsort(cls, kind="stable")
    cls_sorted = cls[order]
    start_sorted = norm_start[order]
    len_sorted = norm_len[order]

    # per-class, per-core slot lists (start, len); dummies = (0, 0)
    core_slots = {c: [[] for _ in range(N_CORES)] for c in CLASSES}
    for c in CLASSES:
        m = cls_sorted == c
        st, ln = start_sorted[m], len_sorted[m]
        n = len(st)
        n_core = int(np.ceil(n / N_CORES)) if n else 0
        for core in range(N_CORES):
            s = st[core::N_CORES]
            l = ln[core::N_CORES]
            pad = n_core - len(s)
            if pad > 0:
                s = np.concatenate([s, np.zeros(pad, np.int64)])
                l = np.concatenate([l, np.zeros(pad, np.int64)])
            core_slots[c][core] = (s, l)

    # --- canonicalize class-48 region: split slots first, then normal 48s;
    #     pad each class region to a multiple of segments-per-group
    groups = []          # (L, n_fix_half0, n_fix_half1) per group
    elem_idx = [[] for _ in range(N_CORES)]
    orig_of = [[] for _ in range(N_CORES)]

    def expand(core, starts, lens, L):
        """Append elements for slots of class L: real then duplicate-padded."""
        starts = np.asarray(starts, np.int64)
        lens = np.asarray(lens, np.int64)
        j = np.arange(L)[None, :]
        last = np.maximum(lens - 1, 0)[:, None]
        idx = starts[:, None] + np.minimum(j, last)
        org = np.where(j < lens[:, None], starts[:, None] + j, -1)
        elem_idx[core].append(idx.ravel())
        orig_of[core].append(org.ravel())

    for c in CLASSES:
        # assemble slot lists (with split pieces at head of class 48)
        per_core_slots = []
        for core in range(N_CORES):
            s, l = core_slots[c][core]
            if c == LMAXC and p_max > 1:
                ss = sp_start[core::N_CORES]
                sl = sp_len[core::N_CORES]
                pad = n_split_core - len(ss)
                if pad > 0:
                    ss = np.concatenate([ss, np.zeros(pad, np.int64)])
                    sl = np.concatenate([sl, np.zeros(pad, np.int64)])
                # expand each split segment into p_max pieces of class 48;
                # empty pieces get len 0 (gather repeats the segment's first
                # element, orig stays -1)
                pst, pln = [], []
                for k in range(p_max):
                    off = k * LMAXC
                    pl = np.clip(sl - off, 0, LMAXC)
                    ps = np.where(pl > 0, ss + off, ss)
                    pst.append(ps)
                    pln.append(pl)
                # seg0 pieces adjacent, then seg1 pieces, ...
                pst = np.stack(pst, 1).ravel()
                pln = np.stack(pln, 1).ravel()
                s = np.concatenate([pst, s])
                l = np.concatenate([pln, l])
            per_core_slots.append((s, l))

        n_slots = max(len(s) for s, _ in per_core_slots)
        seg_per_group = GROUP_COLS // c
        n_slots = int(np.ceil(n_slots / seg_per_group)) * seg_per_group if n_slots else 0
        if n_slots == 0:
            continue
        for core in range(N_CORES):
            s, l = per_core_slots[core]
            pad = n_slots - len(s)
            if pad > 0:
                s = np.concatenate([s, np.zeros(pad, np.int64)])
                l = np.concatenate([l, np.zeros(pad, np.int64)])
            expand(core, s, l, c)
        n_groups = n_slots // seg_per_group
        # fix-up bookkeeping for split pieces (class 48 head)
        total_fix_slots = n_split_core * p_max if (c == LMAXC and p_max > 1) else 0
        spc = FD // c  # slots per chunk(half)
        for g in range(n_groups):
            nfix = [0, 0]
            for h in range(2):
                lo = (g * 2 + h) * spc
                hi = lo + spc
                nf = min(max(total_fix_slots - lo, 0), spc)
                nfix[h] = nf // p_max
            groups.append((c, nfix[0], nfix[1]))

    for core in range(N_CORES):
        elem_idx[core] = np.concatenate(elem_idx[core])
        orig_of[core] = np.concatenate(orig_of[core])
    elem_idx = np.stack(elem_idx)
    orig_of = np.stack(orig_of)

    E = elem_idx.shape[1]
    assert E % GROUP_COLS == 0 and E // GROUP_COLS == len(groups)
    return dict(elem_idx=elem_idx, orig_of=orig_of, groups=groups,
                p_max=p_max, E=E)


# ----------------------------------------------------------------------------
# Device program
# ----------------------------------------------------------------------------

def build_nc(groups, p_max, E, loop_n=1):
    nc = bacc.Bacc("TRN2", target_bir_lowering=False, debug=False)

    xin = nc.declare_dram_parameter("xin", [D_IN, E], F16, isOutput=False)
    out = nc.declare_dram_parameter("out", [D_OUT, E], F32, isOutput=True)
    wnames = ["w11", "w12", "w21", "w22", "w31a", "w31b", "w32"]
    wdims = [D_IN, D_OUT, D_OUT, D_OUT, D_OUT, D_OUT, D_OUT]
    wp = {n: nc.declare_dram_parameter(n, [k, D_OUT], F16, isOutput=False)
          for n, k in zip(wnames, wdims)}
    bnames = ["b11", "b12", "b21", "b22", "b31", "b32"]
    bp = {n: nc.declare_dram_parameter(n, [D_OUT, 1], F32, isOutput=False)
          for n in bnames}

    PR = mybir.ActivationFunctionType.Prelu
    SPAN = GAP + FD          # 992: full group span incl. gap

    with tile.TileContext(nc) as tc:
        with (
            tc.tile_pool(name="wpool", bufs=1) as wpool,
            tc.tile_pool(name="xpool", bufs=4) as xpool,
            tc.tile_pool(name="apool", bufs=3) as apool,
            tc.tile_pool(name="opool", bufs=3) as opool,
            tc.tile_pool(name="ps", bufs=3, space="PSUM") as psp,
            tc.tile_pool(name="pset", bufs=1, space="PSUM") as psq,
        ):
            wt = {}
            for n, k in zip(wnames, wdims):
                wt[n] = wpool.tile([k, D_OUT], F16, tag=f"w_{n}", name=f"w_{n}")
                nc.gpsimd.dma_start(wt[n][:], wp[n][:])
            bt = {}
            for n in bnames:
                bt[n] = wpool.tile([D_OUT, 1], F32, tag=f"b_{n}", name=f"b_{n}")
                nc.gpsimd.dma_start(bt[n][:], bp[n][:])

            import contextlib
            loop_ctx = (tc.For_i(0, loop_n, 1) if loop_n > 1
                        else contextlib.nullcontext())
            with loop_ctx:
                body(nc, tc, groups, p_max, wt, bt,
                     xin, out, xpool, apool, opool, psp, psq, PR, SPAN)

    nc.finalize()
    return nc


def body(nc, tc, groups, p_max, wt, bt, xin, out,
         xpool, apool, opool, psp, psq, PR, SPAN):
            for g, (L, nfix0, nfix1) in enumerate(groups):
                m = FD // L
                xcol = g * GROUP_COLS

                xt = xpool.tile([D_IN, GROUP_COLS], F16, tag="xt")
                nc.sync.dma_start(xt[:], xin[:, xcol:xcol + GROUP_COLS])

                def halves():
                    return ((0, 0), (GAP, FD))  # (psum/sbuf offset, x offset)

                # ---- L1
                u1 = psp.tile([D_OUT, SPAN], F32, tag="ubig")
                for off, xo in halves():
                    nc.tensor.matmul(u1[:, off:off + FD], wt["w11"][:],
                                     xt[:, xo:xo + FD], start=True, stop=True)
                a1 = apool.tile([D_OUT, SPAN], F16, tag="a1")
                nc.scalar.activation(a1[:], u1[:], PR, bias=bt["b11"][:],
                                     scale=1.0, alpha=ALPHA)

                # ---- L2
                u2 = psp.tile([D_OUT, SPAN], F32, tag="ubig")
                for off, _ in halves():
                    nc.tensor.matmul(u2[:, off:off + FD], wt["w12"][:],
                                     a1[:, off:off + FD], start=True, stop=True)
                a2 = apool.tile([D_OUT, SPAN], F16, tag="a2")
                nc.scalar.activation(a2[:], u2[:], PR, bias=bt["b12"][:],
                                     scale=1.0, alpha=ALPHA)

                # ---- segment max
                if L == 1:
                    pooled = a2
                else:
                    pooled = apool.tile([D_OUT, GAP + m], F16, tag="pooled")
                    for h, (off, _) in enumerate(halves()):
                        nc.vector.tensor_reduce(
                            pooled[:, off:off + m],
                            a2[:, off:off + FD].rearrange(
                                "p (m l) -> p m l", m=m, l=L),
                            axis=mybir.AxisListType.X, op=mybir.AluOpType.max)
                    # split-segment second-level fix
                    for h, nfix in ((0, nfix0), (1, nfix1)):
                        if nfix == 0:
                            continue
                        off = h * GAP
                        tmp = apool.tile([D_OUT, nfix], F16, tag="fixtmp")
                        nc.vector.tensor_reduce(
                            tmp[:],
                            pooled[:, off:off + nfix * p_max].rearrange(
                                "p (k q) -> p k q", k=nfix, q=p_max),
                            axis=mybir.AxisListType.X, op=mybir.AluOpType.max)
                        nc.vector.tensor_copy(
                            pooled[:, off:off + nfix * p_max].rearrange(
                                "p (k q) -> p k q", k=nfix, q=p_max),
                            tmp[:].unsqueeze(2).broadcast_to(
                                [D_OUT, nfix, p_max]))

                # ---- mlp_set on segments
                u3 = psq.tile([D_OUT, GAP + m], F32, tag="uset")
                for off, _ in halves():
                    nc.tensor.matmul(u3[:, off:off + m], wt["w21"][:],
                                     pooled[:, off:off + m],
                                     start=True, stop=True)
                a3 = apool.tile([D_OUT, GAP + m], F16, tag="a3")
                nc.scalar.activation(a3[:, :GAP + m], u3[:, :GAP + m], PR,
                                     bias=bt["b21"][:], scale=1.0, alpha=ALPHA)
                u4 = psq.tile([D_OUT, GAP + m], F32, tag="uset")
                for off, _ in halves():
                    nc.tensor.matmul(u4[:, off:off + m], wt["w22"][:],
                                     a3[:, off:off + m], start=True, stop=True)
                a4 = apool.tile([D_OUT, GAP + m], F16, tag="a4")
                nc.scalar.activation(a4[:, :GAP + m], u4[:, :GAP + m], PR,
                                     bias=bt["b22"][:], scale=1.0, alpha=ALPHA)

                # ---- mlp3 layer 1: concat(x1, x_set_e) via two matmuls
                u5 = psp.tile([D_OUT, SPAN], F32, tag="ubig")
                for off, _ in halves():
                    nc.tensor.matmul(u5[:, off:off + FD], wt["w31a"][:],
                                     a2[:, off:off + FD], start=True, stop=False)
                    if L == 1:
                        rhs = a4[:, off:off + FD]
                    else:
                        rhs = a4[:, off:off + m].unsqueeze(2).broadcast_to(
                            [D_OUT, m, L])
                    nc.tensor.matmul(u5[:, off:off + FD], wt["w31b"][:],
                                     rhs, start=False, stop=True)
                a5 = apool.tile([D_OUT, SPAN], F16, tag="a5")
                nc.scalar.activation(a5[:], u5[:], PR, bias=bt["b31"][:],
                                     scale=1.0, alpha=ALPHA)

                # ---- mlp3 layer 2 + fp32 output
                u6 = psp.tile([D_OUT, SPAN], F32, tag="ubig")
                for off, _ in halves():
                    nc.tensor.matmul(u6[:, off:off + FD], wt["w32"][:],
                                     a5[:, off:off + FD], start=True, stop=True)
                ot = opool.tile([D_OUT, 2 * GAP], F32, tag="ot")
                nc.scalar.activation(ot[:, :SPAN], u6[:], PR, bias=bt["b32"][:],
                                     scale=1.0, alpha=ALPHA)
                nc.sync.dma_start(
                    out[:, xcol:xcol + GROUP_COLS],
                    ot[:].rearrange("p (h f) -> p h f", h=2, f=GAP)[:, :, :FD])


# ----------------------------------------------------------------------------
# Entry point
# ----------------------------------------------------------------------------

_CACHE = {}


def prepare(x, csr_idx, w11, s11, b11, w12, s12, b12,
            w21, s21, b21, w22, s22, b22,
            w31, s31, b31, w32, s32, b32, loop_n=1):
    """Build (nc, in_maps, layout) for the given inputs; cached by structure."""
    x = np.asarray(x)
    lay = build_layout(csr_idx)
    E = lay["E"]

    key = (tuple(lay["groups"]), lay["p_max"], E, loop_n)
    if key not in _CACHE:
        _CACHE[key] = build_nc(lay["groups"], lay["p_max"], E, loop_n=loop_n)
    nc = _CACHE[key]

    # fold BN scale into weights, cast fp16
    def wprep(w, s):
        return (np.asarray(w) * np.asarray(s)[None, :]).astype(np.float16)

    w11f = wprep(w11, s11)
    w12f = wprep(w12, s12)
    w21f = wprep(w21, s21)
    w22f = wprep(w22, s22)
    w31f = wprep(w31, s31)
    w32f = wprep(w32, s32)
    params = {
        "w11": w11f, "w12": w12f, "w21": w21f, "w22": w22f,
        "w31a": np.ascontiguousarray(w31f[:D_OUT]),
        "w31b": np.ascontiguousarray(w31f[D_OUT:]),
        "w32": w32f,
        "b11": np.asarray(b11, np.float32).reshape(D_OUT, 1),
        "b12": np.asarray(b12, np.float32).reshape(D_OUT, 1),
        "b21": np.asarray(b21, np.float32).reshape(D_OUT, 1),
        "b22": np.asarray(b22, np.float32).reshape(D_OUT, 1),
        "b31": np.asarray(b31, np.float32).reshape(D_OUT, 1),
        "b32": np.asarray(b32, np.float32).reshape(D_OUT, 1),
    }

    x16 = x.astype(np.float16)
    in_maps = []
    for core in range(N_CORES):
        xc = np.ascontiguousarray(x16[lay["elem_idx"][core]].T)
        in_maps.append({"xin": xc, **params})
    return nc, in_maps, lay


def run_device(nc, in_maps):
    return run_bass_kernel_spmd(nc, in_maps, list(range(N_CORES)))


def postprocess(res, lay, n):
    outp = np.empty((n, D_OUT), np.float32)
    filled = np.zeros(n, bool)
    for core in range(N_CORES):
        o = lay["orig_of"][core]
        m = o >= 0
        outp[o[m]] = res.results[core]["out"][:, m].T
        filled[o[m]] = True
    assert filled.all(), f"missing {int((~filled).sum())} elements"
    return outp


def kernel(x, csr_idx, **kw):
    x = np.asarray(x)
    nc, in_maps, lay = prepare(x, csr_idx, **kw)
    res = run_device(nc, in_maps)
    return postprocess(res, lay, x.shape[0])

